# revision 7
# baseline (speedup 1.0000x reference)
"""Trainium2 Bass kernel for nn_BasicConv2d (int8 conv + global requant + BN + requant + ReLU).

Self-contained: takes full inputs, shards batch dim over 8 NeuronCores,
runs one SPMD Bass program (conv as 9 shifted matmuls, tiny AllGathers for
the global max / BN-stat reductions), gathers full output.
"""
import numpy as np
import ml_dtypes

import jax  # noqa: F401  (axon PJRT backend provides the 8 NeuronCores)

try:
    jax.config.update("jax_compilation_cache_dir", "/tmp/jaxcache")
    jax.config.update("jax_persistent_cache_min_compile_time_secs", 0.0)
except Exception:
    pass

import concourse.bass as bass
import concourse.tile as tile
from concourse import mybir, bacc
from concourse.bass_utils import run_bass_kernel_spmd

F32 = mybir.dt.float32
I32 = mybir.dt.int32
I8 = mybir.dt.int8
BF16 = mybir.dt.bfloat16
AF = mybir.ActivationFunctionType
OP = mybir.AluOpType
AX = mybir.AxisListType

N, CIN, H, W = 32, 128, 56, 56
COUT, KH, KW = 256, 3, 3
OH, OW = 54, 54
PX = OH * OW            # 2916
NCORES = 8
NIMG = N // NCORES      # 4 images per core
NRB = 6                 # row blocks per image (9 output rows each)
RBPX = PX // NRB        # 486 = 9 rows * 54 cols
HALFS = 2               # two 128-channel halves of COUT
COLS_H = NIMG * PX      # 11664 columns per half
COLS = HALFS * COLS_H   # 23328
EPS = 1e-5
RG = [list(range(NCORES))]
CC1 = 520               # [0:256)=chmax, [256:512)=chmin, [512]=local r1
# phase-3 column split per (half, image) chunk of 2916
P3_ACT, P3_DVE, P3_GPS = 1600, 816, 500

_cached = {}


def _col(h, i, rb=0):
    return (h * NIMG + i) * PX + rb * RBPX


def _bitexp_pow2(nc, pool, r_ap, name, p=128):
    """r [p,1] f32 (>0) -> (s [p,1] f32 = 2^(7-ceil(log2 r)),
    bwb [p,1] i32 = ceil(log2 r) + 127). Exact bit arithmetic."""
    ri = r_ap.bitcast(I32)
    eb = pool.tile([p, 1], I32, tag=f"{name}_eb")
    nc.vector.tensor_scalar(eb[:], ri, 23, 0xFF, OP.logical_shift_right, OP.bitwise_and)
    mant = pool.tile([p, 1], I32, tag=f"{name}_mant")
    nc.vector.tensor_scalar(mant[:], ri, 0x7FFFFF, None, OP.bitwise_and)
    nz = pool.tile([p, 1], I32, tag=f"{name}_nz")
    nc.vector.tensor_scalar(nz[:], mant[:], 0, None, OP.is_gt)
    bwb = pool.tile([p, 1], I32, tag=f"{name}_bwb")
    nc.vector.tensor_tensor(bwb[:], eb[:], nz[:], OP.add)
    t = pool.tile([p, 1], I32, tag=f"{name}_t")
    nc.vector.tensor_scalar(t[:], bwb[:], -1, 261, OP.mult, OP.add)  # 261 - bwb
    sb = pool.tile([p, 1], I32, tag=f"{name}_sb")
    nc.vector.tensor_scalar(sb[:], t[:], 23, None, OP.logical_shift_left)
    s = pool.tile([p, 1], F32, tag=f"{name}_s")
    nc.vector.tensor_copy(s[:], sb[:].bitcast(F32))
    return s, bwb


def _pow2_from_int(nc, pool, oi_ap, name, p=128):
    """2^k for k given as [p,1] int32 (normal range)."""
    b = pool.tile([p, 1], I32, tag=f"{name}_b")
    nc.vector.tensor_scalar(b[:], oi_ap, 127, None, OP.add)
    bs = pool.tile([p, 1], I32, tag=f"{name}_bs")
    nc.vector.tensor_scalar(bs[:], b[:], 23, None, OP.logical_shift_left)
    pt = pool.tile([p, 1], F32, tag=f"{name}_p")
    nc.vector.tensor_copy(pt[:], bs[:].bitcast(F32))
    return pt


def _build():
    nc = bacc.Bacc("TRN2", target_bir_lowering=False, debug=False, num_devices=NCORES)

    x_in = nc.dram_tensor("x", [NIMG, CIN, H * W], BF16, kind="ExternalInput")
    w_in = nc.dram_tensor("w", [KH * KW, CIN, COUT], BF16, kind="ExternalInput")
    scal_in = nc.dram_tensor("scal", [1, 1], F32, kind="ExternalInput")  # x_exp+w_exp
    gamma_in = nc.dram_tensor("gamma2", [HALFS, 128], F32, kind="ExternalInput")
    beta_in = nc.dram_tensor("beta2", [HALFS, 128], F32, kind="ExternalInput")
    out_val = nc.dram_tensor("out_val", [NIMG, COUT, PX], I8, kind="ExternalOutput")
    out_exp = nc.dram_tensor("out_exp", [1, 1], F32, kind="ExternalOutput")

    with tile.TileContext(nc) as tc:
        with (
            tc.tile_pool(name="big", bufs=1) as big,
            tc.tile_pool(name="stat", bufs=1) as stat,
            tc.tile_pool(name="dram", bufs=1, space="DRAM") as dram,
            tc.tile_pool(name="psum", bufs=8, space="PSUM") as psum_pool,
        ):
            # ---- PE warmup: dummy matmuls on never-written SBUF, overlaps input DMA
            dummy = big.tile([128, 128], BF16)
            nc.vector.memset(dummy[:], 1.0)
            wps = psum_pool.tile([128, 128], F32, tag="ps")
            for _ in range(36):
                nc.tensor.matmul(wps[:], dummy[:], dummy[:], start=True, stop=True)

            # ---- load inputs to SBUF ----
            x_sb = big.tile([128, NIMG, H * W], BF16)
            for i in range(NIMG):
                nc.sync.dma_start(x_sb[:, i, :], x_in[i])
            w_sb = big.tile([128, KH * KW, COUT], BF16)
            nc.sync.dma_start(w_sb[:], w_in[:].rearrange("k p c -> p k c"))
            gam_sb = stat.tile([128, HALFS], F32)
            nc.sync.dma_start(gam_sb[:], gamma_in[:].rearrange("h p -> p h"))
            bet_sb = stat.tile([128, HALFS], F32)
            nc.sync.dma_start(bet_sb[:], beta_in[:].rearrange("h p -> p h"))
            scal_sb = stat.tile([128, 1], F32)
            nc.sync.dma_start(
                scal_sb[:],
                bass.AP(tensor=scal_in, offset=0, ap=[[0, 128], [1, 1]]),
            )

            acc_sb = big.tile([128, COLS], F32)
            q_sb = big.tile([128, COLS], I8)
            o_sb = big.tile([128, COLS], I8)

            mx_raw = stat.tile([128, HALFS, NIMG * NRB], F32)
            mn_raw = stat.tile([128, HALFS, NIMG * NRB], F32)

            # ---- phase 1: conv (9 shifted matmuls per psum tile) ----
            for i in range(NIMG):
                x_img = x_sb[:, i, :].rearrange("p (r c) -> p r c", c=W)
                for rb in range(NRB):
                    for h in range(HALFS):
                        ps = psum_pool.tile([128, RBPX], F32)
                        for k in range(KH * KW):
                            kh, kw = divmod(k, KW)
                            rhs = x_img[:, rb * 9 + kh : rb * 9 + kh + 9, kw : kw + OW]
                            nc.tensor.matmul(
                                ps[:],
                                w_sb[:, k, h * 128 : (h + 1) * 128],
                                rhs,
                                start=(k == 0),
                                stop=(k == KH * KW - 1),
                            )
                        c0 = _col(h, i, rb)
                        nc.scalar.activation(acc_sb[:, c0 : c0 + RBPX], ps[:], AF.Copy)
                        j = i * NRB + rb
                        nc.vector.tensor_reduce(
                            mx_raw[:, h, j : j + 1], ps[:], AX.X, OP.max
                        )
                        nc.vector.tensor_reduce(
                            mn_raw[:, h, j : j + 1], ps[:], AX.X, OP.min
                        )

            # per-core per-channel acc max/min
            chmax = stat.tile([128, HALFS], F32)
            nc.vector.tensor_reduce(chmax[:], mx_raw[:], AX.X, OP.max)
            chmin = stat.tile([128, HALFS], F32)
            nc.vector.tensor_reduce(chmin[:], mn_raw[:], AX.X, OP.min)

            # ---- collective 1: AllGather per-channel acc max/min + local r1 ----
            cc1_in = dram.tile([CC1], F32)
            for h in range(HALFS):
                nc.sync.dma_start(cc1_in[h * 128 : (h + 1) * 128], chmax[:, h : h + 1])
                nc.sync.dma_start(
                    cc1_in[256 + h * 128 : 256 + (h + 1) * 128], chmin[:, h : h + 1]
                )
            # local r1 scalar: cross-partition absmax via a single-row read of
            # the packed buffer (hidden in the pre-collective window)
            rl2 = stat.tile([1, 512], F32)
            nc.sync.dma_start(
                rl2[:],
                bass.AP(tensor=cc1_in.tensor, offset=cc1_in[:].offset,
                        ap=[[0, 1], [1, 512]]),
            )
            r1c = stat.tile([1, 1], F32)
            nc.vector.tensor_reduce(r1c[:], rl2[:], AX.X, OP.max,
                                    apply_absolute_value=True)
            nc.sync.dma_start(cc1_in[512:513], r1c[:])

            cc1_out = dram.tile([NCORES, CC1], F32)
            nc.gpsimd.collective_compute(
                "AllGather", OP.bypass, replica_groups=RG,
                ins=[cc1_in[:].opt()], outs=[cc1_out[:].opt()],
            )

            # r1 on all partitions: broadcast-read the 8 per-core r1 slots
            r1g = stat.tile([128, NCORES], F32)
            nc.sync.dma_start(
                r1g[:],
                bass.AP(tensor=cc1_out.tensor, offset=cc1_out[:].offset + 512,
                        ap=[[0, 128], [CC1, NCORES]]),
            )
            r1 = stat.tile([128, 1], F32)
            nc.vector.tensor_reduce(r1[:], r1g[:], AX.X, OP.max,
                                    apply_absolute_value=True)
            r1m = stat.tile([128, 1], F32)
            nc.vector.tensor_scalar(r1m[:], r1[:], 1e-30, None, OP.max)
            s1_b, bwb1 = _bitexp_pow2(nc, stat, r1m[:], "s1")

            # oe = (x_exp+w_exp) + bw1 - 7 ;  poe = 2^oe, poe2 = 2^(2*oe)
            sxw_i = stat.tile([128, 1], I32)
            nc.vector.tensor_copy(sxw_i[:], scal_sb[:])
            oe_i = stat.tile([128, 1], I32)
            nc.vector.tensor_scalar(oe_i[:], bwb1[:], 1, -134, OP.mult, OP.add)
            nc.vector.tensor_tensor(oe_i[:], oe_i[:], sxw_i[:], OP.add)
            oe2_i = stat.tile([128, 1], I32)
            nc.vector.tensor_scalar(oe2_i[:], oe_i[:], 2, None, OP.mult)
            poe_b = _pow2_from_int(nc, stat, oe_i[:], "poe")
            poe2_b = _pow2_from_int(nc, stat, oe2_i[:], "poe2")

            # global per-channel acc extremes -> q extremes
            gmax = stat.tile([128, HALFS, NCORES], F32)
            gmin = stat.tile([128, HALFS, NCORES], F32)
            for h in range(HALFS):
                nc.sync.dma_start(
                    gmax[:, h, :],
                    bass.AP(tensor=cc1_out.tensor,
                            offset=cc1_out[:].offset + h * 128,
                            ap=[[1, 128], [CC1, NCORES]]),
                )
                nc.sync.dma_start(
                    gmin[:, h, :],
                    bass.AP(tensor=cc1_out.tensor,
                            offset=cc1_out[:].offset + 256 + h * 128,
                            ap=[[1, 128], [CC1, NCORES]]),
                )
            gchmax = stat.tile([128, HALFS], F32)
            nc.vector.tensor_reduce(gchmax[:], gmax[:], AX.X, OP.max)
            gchmin = stat.tile([128, HALFS], F32)
            nc.vector.tensor_reduce(gchmin[:], gmin[:], AX.X, OP.min)
            qmx8 = stat.tile([128, HALFS], I8)
            nc.scalar.activation(qmx8[:], gchmax[:], AF.Copy, scale=s1_b[:, 0:1])
            qmn8 = stat.tile([128, HALFS], I8)
            nc.scalar.activation(qmn8[:], gchmin[:], AF.Copy, scale=s1_b[:, 0:1])
            qmaxf = stat.tile([128, HALFS], F32)
            nc.vector.tensor_copy(qmaxf[:], qmx8[:])
            qminf = stat.tile([128, HALFS], F32)
            nc.vector.tensor_copy(qminf[:], qmn8[:])

            # ---- phase 2: q = int8(acc * s1) ; bn stats of q ----
            stats6 = stat.tile([128, HALFS, NIMG * NRB, 6], F32)
            for h in range(HALFS):
                for i in range(NIMG):
                    c0 = _col(h, i)
                    nc.scalar.activation(
                        q_sb[:, c0 : c0 + PX], acc_sb[:, c0 : c0 + PX],
                        AF.Copy, scale=s1_b[:, 0:1],
                    )
                    for rb in range(NRB):
                        cb = c0 + rb * RBPX
                        nc.vector.bn_stats(
                            stats6[:, h, i * NRB + rb, :], q_sb[:, cb : cb + RBPX]
                        )
            mv = stat.tile([128, HALFS, 2], F32)
            for h in range(HALFS):
                nc.vector.bn_aggr(mv[:, h, :], stats6[:, h, :, :])

            # ---- collective 2: AllGather per-channel (mean, var) ----
            cc2_in = dram.tile([2 * HALFS * 128], F32)
            for h in range(HALFS):
                nc.sync.dma_start(cc2_in[h * 128 : (h + 1) * 128], mv[:, h, 0:1])
                nc.sync.dma_start(
                    cc2_in[256 + h * 128 : 256 + (h + 1) * 128], mv[:, h, 1:2]
                )
            cc2_out = dram.tile([NCORES, 2 * HALFS * 128], F32)
            nc.gpsimd.collective_compute(
                "AllGather", OP.bypass, replica_groups=RG,
                ins=[cc2_in[:].opt()], outs=[cc2_out[:].opt()],
            )
            gmean = stat.tile([128, HALFS, NCORES], F32)
            gvar = stat.tile([128, HALFS, NCORES], F32)
            for h in range(HALFS):
                nc.sync.dma_start(
                    gmean[:, h, :],
                    bass.AP(tensor=cc2_out.tensor,
                            offset=cc2_out[:].offset + h * 128,
                            ap=[[1, 128], [2 * HALFS * 128, NCORES]]),
                )
                nc.sync.dma_start(
                    gvar[:, h, :],
                    bass.AP(tensor=cc2_out.tensor,
                            offset=cc2_out[:].offset + 256 + h * 128,
                            ap=[[1, 128], [2 * HALFS * 128, NCORES]]),
                )

            # combine: mean_g = avg(mean_i); var_g = avg(var_i + mean_i^2) - mean_g^2
            mean_g = stat.tile([128, HALFS], F32)
            nc.vector.tensor_reduce(mean_g[:], gmean[:], AX.X, OP.add)
            nc.vector.tensor_scalar(mean_g[:], mean_g[:], 1.0 / NCORES, None, OP.mult)
            m2t = stat.tile([128, HALFS, NCORES], F32)
            nc.vector.tensor_tensor(m2t[:], gmean[:], gmean[:], OP.mult)
            nc.vector.tensor_tensor(m2t[:], m2t[:], gvar[:], OP.add)
            ex2 = stat.tile([128, HALFS], F32)
            nc.vector.tensor_reduce(ex2[:], m2t[:], AX.X, OP.add)
            nc.vector.tensor_scalar(ex2[:], ex2[:], 1.0 / NCORES, None, OP.mult)
            var_g = stat.tile([128, HALFS], F32)
            nc.vector.tensor_tensor(var_g[:], mean_g[:], mean_g[:], OP.mult)
            nc.vector.tensor_tensor(var_g[:], ex2[:], var_g[:], OP.subtract)

            # rs = rsqrt(var_g * 2^(2oe) + eps), Newton-refined
            v = stat.tile([128, HALFS], F32)
            nc.scalar.activation(v[:], var_g[:], AF.Copy, scale=poe2_b[:, 0:1])
            veps = stat.tile([128, HALFS], F32)
            nc.vector.tensor_scalar(veps[:], v[:], EPS, None, OP.add)
            eps_t = stat.tile([128, 1], F32)
            nc.vector.memset(eps_t[:], EPS)
            s_sq = stat.tile([128, HALFS], F32)
            nc.scalar.activation(s_sq[:], v[:], AF.Sqrt, bias=eps_t[:, 0:1])
            for it in range(2):
                rcp = stat.tile([128, HALFS], F32, tag=f"rcp{it}")
                nc.vector.reciprocal(rcp[:], s_sq[:])
                tn = stat.tile([128, HALFS], F32, tag=f"tn{it}")
                nc.vector.tensor_tensor(tn[:], veps[:], rcp[:], OP.mult)
                nc.vector.tensor_tensor(tn[:], tn[:], s_sq[:], OP.add)
                nc.vector.tensor_scalar(s_sq[:], tn[:], 0.5, None, OP.mult)
            rs = stat.tile([128, HALFS], F32)
            nc.vector.reciprocal(rs[:], s_sq[:])

            # A0 = 2^oe * rs * gamma ; B0 = beta - mean_g*2^oe * rs*gamma
            rg_t = stat.tile([128, HALFS], F32)
            nc.vector.tensor_tensor(rg_t[:], rs[:], gam_sb[:], OP.mult)
            a0 = stat.tile([128, HALFS], F32)
            nc.scalar.activation(a0[:], rg_t[:], AF.Copy, scale=poe_b[:, 0:1])
            mq = stat.tile([128, HALFS], F32)
            nc.scalar.activation(mq[:], mean_g[:], AF.Copy, scale=poe_b[:, 0:1])
            u = stat.tile([128, HALFS], F32)
            nc.vector.tensor_tensor(u[:], mq[:], rg_t[:], OP.mult)
            b0 = stat.tile([128, HALFS], F32)
            nc.vector.tensor_tensor(b0[:], bet_sb[:], u[:], OP.subtract)

            # r2 = max_c max(|A0*qmax+B0|, |A0*qmin+B0|)
            c1 = stat.tile([128, HALFS], F32)
            nc.vector.tensor_tensor(c1[:], a0[:], qmaxf[:], OP.mult)
            nc.vector.tensor_tensor(c1[:], c1[:], b0[:], OP.add)
            nc.scalar.activation(c1[:], c1[:], AF.Abs)
            c2 = stat.tile([128, HALFS], F32)
            nc.vector.tensor_tensor(c2[:], a0[:], qminf[:], OP.mult)
            nc.vector.tensor_tensor(c2[:], c2[:], b0[:], OP.add)
            nc.scalar.activation(c2[:], c2[:], AF.Abs)
            chr2 = stat.tile([128, HALFS], F32)
            nc.vector.tensor_tensor(chr2[:], c1[:], c2[:], OP.max)
            rr2 = stat.tile([128, 1], F32)
            nc.vector.tensor_tensor(rr2[:], chr2[:, 0:1], chr2[:, 1:2], OP.max)
            r2col = dram.tile([128], F32)
            nc.sync.dma_start(r2col[:], rr2[:])
            r2all = stat.tile([128, 128], F32)
            nc.sync.dma_start(
                r2all[:],
                bass.AP(tensor=r2col.tensor, offset=r2col[:].offset,
                        ap=[[0, 128], [1, 128]]),
            )
            r2 = stat.tile([128, 1], F32)
            nc.vector.tensor_reduce(r2[:], r2all[:], AX.X, OP.max)
            r2m = stat.tile([128, 1], F32)
            nc.vector.tensor_scalar(r2m[:], r2[:], 1e-30, None, OP.max)
            s2_b, bwb2 = _bitexp_pow2(nc, stat, r2m[:], "s2")

            # exp2 = bw2 - 7
            e2i = stat.tile([128, 1], I32)
            nc.vector.tensor_scalar(e2i[:], bwb2[:], 1, -134, OP.mult, OP.add)
            e2f = stat.tile([128, 1], F32)
            nc.vector.tensor_copy(e2f[:], e2i[:])
            nc.sync.dma_start(out_exp[:], e2f[0:1, 0:1])

            # A' = A0*s2, B' = B0*s2
            ap_ = stat.tile([128, HALFS], F32)
            nc.scalar.activation(ap_[:], a0[:], AF.Copy, scale=s2_b[:, 0:1])
            bp_ = stat.tile([128, HALFS], F32)
            nc.scalar.activation(bp_[:], b0[:], AF.Copy, scale=s2_b[:, 0:1])

            # ---- phase 3: out = int8(relu(A'*q + B')), split ACT/DVE/GPSIMD ----
            for h in range(HALFS):
                ah, bh = ap_[:, h : h + 1], bp_[:, h : h + 1]
                for i in range(NIMG):
                    c0 = _col(h, i)
                    ca, cd = c0, c0 + P3_ACT
                    cg, ce = cd + P3_DVE, c0 + PX
                    nc.scalar.activation(
                        o_sb[:, ca:cd], q_sb[:, ca:cd], AF.Relu, bias=bh, scale=ah
                    )
                    tmp = acc_sb[:, cd:cg]  # acc is dead; reuse as f32 scratch
                    nc.vector.tensor_scalar(tmp, q_sb[:, cd:cg], ah, bh,
                                            OP.mult, OP.add)
                    nc.vector.tensor_scalar(o_sb[:, cd:cg], tmp, 0.0, None, OP.max)
                    tmpg = acc_sb[:, cg:ce]
                    nc.gpsimd.tensor_scalar(tmpg, q_sb[:, cg:ce], ah, bh,
                                            OP.mult, OP.add)
                    nc.gpsimd.tensor_scalar(o_sb[:, cg:ce], tmpg, 0.0, None, OP.max)
                    nc.sync.dma_start(
                        out_val[i, h * 128 : (h + 1) * 128, :],
                        o_sb[:, c0 : c0 + PX],
                    )

    nc.finalize()
    return nc


def _get_nc():
    if "nc" not in _cached:
        _cached["nc"] = _build()
    return _cached["nc"]


def kernel(x_val, x_exp, w_val, w_exp, gamma, beta, _trace=False):
    nc = _get_nc()

    bf16 = ml_dtypes.bfloat16
    x = np.asarray(x_val).reshape(N, CIN, H * W).astype(bf16)
    # weights: [COUT, CIN, KH, KW] -> [KH*KW, CIN, COUT]
    w = np.ascontiguousarray(
        np.asarray(w_val).astype(np.float32).transpose(2, 3, 1, 0).reshape(KH * KW, CIN, COUT)
    ).astype(bf16)
    sxw = np.array([[np.float32(x_exp) + np.float32(w_exp)]], dtype=np.float32)
    g2 = np.ascontiguousarray(np.asarray(gamma, np.float32).reshape(HALFS, 128))
    b2 = np.ascontiguousarray(np.asarray(beta, np.float32).reshape(HALFS, 128))

    in_maps = []
    for c in range(NCORES):
        in_maps.append({
            "x": np.ascontiguousarray(x[c * NIMG : (c + 1) * NIMG]),
            "w": w,
            "scal": sxw,
            "gamma2": g2,
            "beta2": b2,
        })

    res = run_bass_kernel_spmd(nc, in_maps, list(range(NCORES)), trace=_trace)
    out = np.concatenate([res.results[c]["out_val"] for c in range(NCORES)], axis=0)
    out = out.reshape(N, COUT, OH, OW)
    exp2 = np.float32(res.results[0]["out_exp"][0, 0])
    if _trace:
        kernel.last_results = res
    return out, exp2


# revision 8
# speedup vs baseline: 1.1043x; 1.1043x over previous
"""Trainium2 Bass kernel for nn_BasicConv2d (int8 conv + global requant + BN + requant + ReLU).

Self-contained: takes full inputs, shards batch dim over 8 NeuronCores,
runs one SPMD Bass program (conv as 9 shifted matmuls, tiny AllGathers for
the global max / BN-stat reductions), gathers full output.
"""
import numpy as np
import ml_dtypes

import jax  # noqa: F401  (axon PJRT backend provides the 8 NeuronCores)

try:
    jax.config.update("jax_compilation_cache_dir", "/tmp/jaxcache")
    jax.config.update("jax_persistent_cache_min_compile_time_secs", 0.0)
except Exception:
    pass

import concourse.bass as bass
import concourse.tile as tile
from concourse import mybir, bacc
from concourse.bass_utils import run_bass_kernel_spmd

F32 = mybir.dt.float32
I32 = mybir.dt.int32
I8 = mybir.dt.int8
BF16 = mybir.dt.bfloat16
AF = mybir.ActivationFunctionType
OP = mybir.AluOpType
AX = mybir.AxisListType

N, CIN, H, W = 32, 128, 56, 56
COUT, KH, KW = 256, 3, 3
OH, OW = 54, 54
PX = OH * OW            # 2916
NCORES = 8
NIMG = N // NCORES      # 4 images per core
NRB = 6                 # row blocks per image (9 output rows each)
RBPX = PX // NRB        # 486 = 9 rows * 54 cols
HALFS = 2               # two 128-channel halves of COUT
COLS_H = NIMG * PX      # 11664 columns per half
COLS = HALFS * COLS_H   # 23328
EPS = 1e-5
RG = [list(range(NCORES))]
CC1 = 520               # [0:256)=chmax, [256:512)=chmin, [512]=local r1
# phase-3 column split per (half, image) chunk of 2916
P3_ACT, P3_DVE, P3_GPS = 1600, 816, 500

_cached = {}


def _col(h, i, rb=0):
    return (h * NIMG + i) * PX + rb * RBPX


def _bitexp_pow2(nc, pool, r_ap, name, p=128):
    """r [p,1] f32 (>0) -> (s [p,1] f32 = 2^(7-ceil(log2 r)),
    bwb [p,1] i32 = ceil(log2 r) + 127). Exact bit arithmetic."""
    ri = r_ap.bitcast(I32)
    eb = pool.tile([p, 1], I32, tag=f"{name}_eb")
    nc.vector.tensor_scalar(eb[:], ri, 23, 0xFF, OP.logical_shift_right, OP.bitwise_and)
    mant = pool.tile([p, 1], I32, tag=f"{name}_mant")
    nc.vector.tensor_scalar(mant[:], ri, 0x7FFFFF, None, OP.bitwise_and)
    nz = pool.tile([p, 1], I32, tag=f"{name}_nz")
    nc.vector.tensor_scalar(nz[:], mant[:], 0, None, OP.is_gt)
    bwb = pool.tile([p, 1], I32, tag=f"{name}_bwb")
    nc.vector.tensor_tensor(bwb[:], eb[:], nz[:], OP.add)
    t = pool.tile([p, 1], I32, tag=f"{name}_t")
    nc.vector.tensor_scalar(t[:], bwb[:], -1, 261, OP.mult, OP.add)  # 261 - bwb
    sb = pool.tile([p, 1], I32, tag=f"{name}_sb")
    nc.vector.tensor_scalar(sb[:], t[:], 23, None, OP.logical_shift_left)
    s = pool.tile([p, 1], F32, tag=f"{name}_s")
    nc.vector.tensor_copy(s[:], sb[:].bitcast(F32))
    return s, bwb


def _pow2_from_int(nc, pool, oi_ap, name, p=128):
    """2^k for k given as [p,1] int32 (normal range)."""
    b = pool.tile([p, 1], I32, tag=f"{name}_b")
    nc.vector.tensor_scalar(b[:], oi_ap, 127, None, OP.add)
    bs = pool.tile([p, 1], I32, tag=f"{name}_bs")
    nc.vector.tensor_scalar(bs[:], b[:], 23, None, OP.logical_shift_left)
    pt = pool.tile([p, 1], F32, tag=f"{name}_p")
    nc.vector.tensor_copy(pt[:], bs[:].bitcast(F32))
    return pt


def _build():
    nc = bacc.Bacc("TRN2", target_bir_lowering=False, debug=False, num_devices=NCORES)

    x_in = nc.dram_tensor("x", [NIMG, CIN, H * W], BF16, kind="ExternalInput")
    w_in = nc.dram_tensor("w", [KH * KW, CIN, COUT], BF16, kind="ExternalInput")
    scal_in = nc.dram_tensor("scal", [1, 1], F32, kind="ExternalInput")  # x_exp+w_exp
    gamma_in = nc.dram_tensor("gamma2", [HALFS, 128], F32, kind="ExternalInput")
    beta_in = nc.dram_tensor("beta2", [HALFS, 128], F32, kind="ExternalInput")
    out_val = nc.dram_tensor("out_val", [NIMG, COUT, PX], I8, kind="ExternalOutput")
    out_exp = nc.dram_tensor("out_exp", [1, 1], F32, kind="ExternalOutput")

    with tile.TileContext(nc) as tc:
        with (
            tc.tile_pool(name="big", bufs=1) as big,
            tc.tile_pool(name="stat", bufs=1) as stat,
            tc.tile_pool(name="dram", bufs=1, space="DRAM") as dram,
            tc.tile_pool(name="psum", bufs=8, space="PSUM") as psum_pool,
        ):
            # ---- PE warmup: dummy matmuls on never-written SBUF, overlaps input DMA
            dummy = big.tile([128, 128], BF16)
            nc.vector.memset(dummy[:], 1.0)
            wps = psum_pool.tile([128, 128], F32, tag="ps")
            for _ in range(36):
                nc.tensor.matmul(wps[:], dummy[:], dummy[:], start=True, stop=True)

            # ---- load inputs to SBUF ----
            x_sb = big.tile([128, NIMG, H * W], BF16)
            for i in range(NIMG):
                nc.sync.dma_start(x_sb[:, i, :], x_in[i])
            w_sb = big.tile([128, KH * KW, COUT], BF16)
            nc.sync.dma_start(w_sb[:], w_in[:].rearrange("k p c -> p k c"))
            gam_sb = stat.tile([128, HALFS], F32)
            nc.sync.dma_start(gam_sb[:], gamma_in[:].rearrange("h p -> p h"))
            bet_sb = stat.tile([128, HALFS], F32)
            nc.sync.dma_start(bet_sb[:], beta_in[:].rearrange("h p -> p h"))
            scal_sb = stat.tile([128, 1], F32)
            nc.sync.dma_start(
                scal_sb[:],
                bass.AP(tensor=scal_in, offset=0, ap=[[0, 128], [1, 1]]),
            )

            acc_sb = big.tile([128, COLS], F32)
            q_sb = big.tile([128, COLS], I8)
            o_sb = big.tile([128, COLS], I8)

            mx_raw = stat.tile([128, HALFS, NIMG * NRB], F32)
            mn_raw = stat.tile([128, HALFS, NIMG * NRB], F32)

            # ---- phase 1: conv (9 shifted matmuls per psum tile) ----
            for i in range(NIMG):
                x_img = x_sb[:, i, :].rearrange("p (r c) -> p r c", c=W)
                for rb in range(NRB):
                    for h in range(HALFS):
                        ps = psum_pool.tile([128, RBPX], F32)
                        for k in range(KH * KW):
                            kh, kw = divmod(k, KW)
                            rhs = x_img[:, rb * 9 + kh : rb * 9 + kh + 9, kw : kw + OW]
                            nc.tensor.matmul(
                                ps[:],
                                w_sb[:, k, h * 128 : (h + 1) * 128],
                                rhs,
                                start=(k == 0),
                                stop=(k == KH * KW - 1),
                            )
                        c0 = _col(h, i, rb)
                        nc.scalar.activation(acc_sb[:, c0 : c0 + RBPX], ps[:], AF.Copy)
                        j = i * NRB + rb
                        nc.vector.tensor_reduce(
                            mx_raw[:, h, j : j + 1], ps[:], AX.X, OP.max
                        )
                        nc.vector.tensor_reduce(
                            mn_raw[:, h, j : j + 1], ps[:], AX.X, OP.min
                        )

            # per-core per-channel acc max/min
            chmax = stat.tile([128, HALFS], F32)
            nc.vector.tensor_reduce(chmax[:], mx_raw[:], AX.X, OP.max)
            chmin = stat.tile([128, HALFS], F32)
            nc.vector.tensor_reduce(chmin[:], mn_raw[:], AX.X, OP.min)

            # ---- collective 1: AllGather per-channel acc max/min + local r1 ----
            cc1_in = dram.tile([CC1], F32)
            for h in range(HALFS):
                nc.sync.dma_start(cc1_in[h * 128 : (h + 1) * 128], chmax[:, h : h + 1])
                nc.sync.dma_start(
                    cc1_in[256 + h * 128 : 256 + (h + 1) * 128], chmin[:, h : h + 1]
                )
            # local r1 scalar: cross-partition absmax via a single-row read of
            # the packed buffer (hidden in the pre-collective window)
            rl2 = stat.tile([1, 512], F32)
            nc.sync.dma_start(
                rl2[:],
                bass.AP(tensor=cc1_in.tensor, offset=cc1_in[:].offset,
                        ap=[[0, 1], [1, 512]]),
            )
            r1c = stat.tile([1, 1], F32)
            nc.vector.tensor_reduce(r1c[:], rl2[:], AX.X, OP.max,
                                    apply_absolute_value=True)
            nc.sync.dma_start(cc1_in[512:513], r1c[:])

            cc1_out = dram.tile([NCORES, CC1], F32)
            nc.gpsimd.collective_compute(
                "AllGather", OP.bypass, replica_groups=RG,
                ins=[cc1_in[:].opt()], outs=[cc1_out[:].opt()],
            )

            # r1 on all partitions: broadcast-read the 8 per-core r1 slots
            r1g = stat.tile([128, NCORES], F32)
            nc.sync.dma_start(
                r1g[:],
                bass.AP(tensor=cc1_out.tensor, offset=cc1_out[:].offset + 512,
                        ap=[[0, 128], [CC1, NCORES]]),
            )
            r1 = stat.tile([128, 1], F32)
            nc.vector.tensor_reduce(r1[:], r1g[:], AX.X, OP.max,
                                    apply_absolute_value=True)
            r1m = stat.tile([128, 1], F32)
            nc.vector.tensor_scalar(r1m[:], r1[:], 1.0, None, OP.max)
            # r1 is integer-valued: ceil(log2 r) = floor(log2(2r-1)) = expfield-127
            t2r = stat.tile([128, 1], F32)
            nc.vector.tensor_scalar(t2r[:], r1m[:], 2.0, -1.0, OP.mult, OP.add)
            bwb1 = stat.tile([128, 1], I32)
            nc.vector.tensor_scalar(bwb1[:], t2r[:].bitcast(I32), 23, 0xFF,
                                    OP.logical_shift_right, OP.bitwise_and)
            s1i = stat.tile([128, 1], I32)
            nc.vector.tensor_scalar(s1i[:], bwb1[:], -1, 261, OP.mult, OP.add)
            nc.vector.tensor_scalar(s1i[:], s1i[:], 23, None, OP.logical_shift_left)
            s1_b = stat.tile([128, 1], F32)
            nc.vector.tensor_copy(s1_b[:], s1i[:].bitcast(F32))

            # oe = (x_exp+w_exp) + bw1 - 7 ;  poe = 2^oe, poe2 = 2^(2*oe)
            sxw_i = stat.tile([128, 1], I32)
            nc.vector.tensor_copy(sxw_i[:], scal_sb[:])
            oe_i = stat.tile([128, 1], I32)
            nc.vector.tensor_scalar(oe_i[:], bwb1[:], 1, -134, OP.mult, OP.add)
            nc.vector.tensor_tensor(oe_i[:], oe_i[:], sxw_i[:], OP.add)
            oe2_i = stat.tile([128, 1], I32)
            nc.vector.tensor_scalar(oe2_i[:], oe_i[:], 2, None, OP.mult)
            poe_b = _pow2_from_int(nc, stat, oe_i[:], "poe")
            poe2_b = _pow2_from_int(nc, stat, oe2_i[:], "poe2")

            # global per-channel acc extremes -> q extremes
            gmax = stat.tile([128, HALFS, NCORES], F32)
            gmin = stat.tile([128, HALFS, NCORES], F32)
            for h in range(HALFS):
                nc.sync.dma_start(
                    gmax[:, h, :],
                    bass.AP(tensor=cc1_out.tensor,
                            offset=cc1_out[:].offset + h * 128,
                            ap=[[1, 128], [CC1, NCORES]]),
                )
                nc.sync.dma_start(
                    gmin[:, h, :],
                    bass.AP(tensor=cc1_out.tensor,
                            offset=cc1_out[:].offset + 256 + h * 128,
                            ap=[[1, 128], [CC1, NCORES]]),
                )
            gchmax = stat.tile([128, HALFS], F32)
            nc.vector.tensor_reduce(gchmax[:], gmax[:], AX.X, OP.max)
            gchmin = stat.tile([128, HALFS], F32)
            nc.vector.tensor_reduce(gchmin[:], gmin[:], AX.X, OP.min)
            qmx8 = stat.tile([128, HALFS], I8)
            nc.scalar.activation(qmx8[:], gchmax[:], AF.Copy, scale=s1_b[:, 0:1])
            qmn8 = stat.tile([128, HALFS], I8)
            nc.scalar.activation(qmn8[:], gchmin[:], AF.Copy, scale=s1_b[:, 0:1])
            qmaxf = stat.tile([128, HALFS], F32)
            nc.vector.tensor_copy(qmaxf[:], qmx8[:])
            qminf = stat.tile([128, HALFS], F32)
            nc.vector.tensor_copy(qminf[:], qmn8[:])

            # ---- phase 2: q = int8(acc * s1) ; bn stats of q ----
            stats6 = stat.tile([128, HALFS, NIMG * NRB, 6], F32)
            for h in range(HALFS):
                for i in range(NIMG):
                    c0 = _col(h, i)
                    nc.scalar.activation(
                        q_sb[:, c0 : c0 + PX], acc_sb[:, c0 : c0 + PX],
                        AF.Copy, scale=s1_b[:, 0:1],
                    )
                    for rb in range(NRB):
                        cb = c0 + rb * RBPX
                        nc.vector.bn_stats(
                            stats6[:, h, i * NRB + rb, :], q_sb[:, cb : cb + RBPX]
                        )
            mv = stat.tile([128, HALFS, 2], F32)
            for h in range(HALFS):
                nc.vector.bn_aggr(mv[:, h, :], stats6[:, h, :, :])

            # ---- collective 2: AllGather per-channel (mean, var) ----
            cc2_in = dram.tile([2 * HALFS * 128], F32)
            for h in range(HALFS):
                nc.sync.dma_start(cc2_in[h * 128 : (h + 1) * 128], mv[:, h, 0:1])
                nc.sync.dma_start(
                    cc2_in[256 + h * 128 : 256 + (h + 1) * 128], mv[:, h, 1:2]
                )
            cc2_out = dram.tile([NCORES, 2 * HALFS * 128], F32)
            nc.gpsimd.collective_compute(
                "AllGather", OP.bypass, replica_groups=RG,
                ins=[cc2_in[:].opt()], outs=[cc2_out[:].opt()],
            )
            gmean = stat.tile([128, HALFS, NCORES], F32)
            gvar = stat.tile([128, HALFS, NCORES], F32)
            for h in range(HALFS):
                nc.sync.dma_start(
                    gmean[:, h, :],
                    bass.AP(tensor=cc2_out.tensor,
                            offset=cc2_out[:].offset + h * 128,
                            ap=[[1, 128], [2 * HALFS * 128, NCORES]]),
                )
                nc.sync.dma_start(
                    gvar[:, h, :],
                    bass.AP(tensor=cc2_out.tensor,
                            offset=cc2_out[:].offset + 256 + h * 128,
                            ap=[[1, 128], [2 * HALFS * 128, NCORES]]),
                )

            # combine: mean_g = avg(mean_i); var_g = avg(var_i + mean_i^2) - mean_g^2
            mean_g = stat.tile([128, HALFS], F32)
            nc.vector.tensor_reduce(mean_g[:], gmean[:], AX.X, OP.add)
            nc.vector.tensor_scalar(mean_g[:], mean_g[:], 1.0 / NCORES, None, OP.mult)
            m2t = stat.tile([128, HALFS, NCORES], F32)
            nc.vector.tensor_tensor(m2t[:], gmean[:], gmean[:], OP.mult)
            nc.vector.tensor_tensor(m2t[:], m2t[:], gvar[:], OP.add)
            ex2 = stat.tile([128, HALFS], F32)
            nc.vector.tensor_reduce(ex2[:], m2t[:], AX.X, OP.add)
            nc.vector.tensor_scalar(ex2[:], ex2[:], 1.0 / NCORES, None, OP.mult)
            var_g = stat.tile([128, HALFS], F32)
            nc.vector.tensor_tensor(var_g[:], mean_g[:], mean_g[:], OP.mult)
            nc.vector.tensor_tensor(var_g[:], ex2[:], var_g[:], OP.subtract)

            # rs = rsqrt(var_g * 2^(2oe) + eps), Newton-refined
            v = stat.tile([128, HALFS], F32)
            nc.scalar.activation(v[:], var_g[:], AF.Copy, scale=poe2_b[:, 0:1])
            veps = stat.tile([128, HALFS], F32)
            nc.vector.tensor_scalar(veps[:], v[:], EPS, None, OP.add)
            eps_t = stat.tile([128, 1], F32)
            nc.vector.memset(eps_t[:], EPS)
            s_sq = stat.tile([128, HALFS], F32)
            nc.scalar.activation(s_sq[:], v[:], AF.Sqrt, bias=eps_t[:, 0:1])
            for it in range(1):
                rcp = stat.tile([128, HALFS], F32, tag=f"rcp{it}")
                nc.vector.reciprocal(rcp[:], s_sq[:])
                tn = stat.tile([128, HALFS], F32, tag=f"tn{it}")
                nc.vector.tensor_tensor(tn[:], veps[:], rcp[:], OP.mult)
                nc.vector.tensor_tensor(tn[:], tn[:], s_sq[:], OP.add)
                nc.vector.tensor_scalar(s_sq[:], tn[:], 0.5, None, OP.mult)
            rs = stat.tile([128, HALFS], F32)
            nc.vector.reciprocal(rs[:], s_sq[:])

            # A0 = 2^oe * rs * gamma ; B0 = beta - mean_g*2^oe * rs*gamma
            rg_t = stat.tile([128, HALFS], F32)
            nc.vector.tensor_tensor(rg_t[:], rs[:], gam_sb[:], OP.mult)
            a0 = stat.tile([128, HALFS], F32)
            nc.scalar.activation(a0[:], rg_t[:], AF.Copy, scale=poe_b[:, 0:1])
            mq = stat.tile([128, HALFS], F32)
            nc.scalar.activation(mq[:], mean_g[:], AF.Copy, scale=poe_b[:, 0:1])
            u = stat.tile([128, HALFS], F32)
            nc.vector.tensor_tensor(u[:], mq[:], rg_t[:], OP.mult)
            b0 = stat.tile([128, HALFS], F32)
            nc.vector.tensor_tensor(b0[:], bet_sb[:], u[:], OP.subtract)

            # r2 = max_c max(|A0*qmax+B0|, |A0*qmin+B0|)
            c1 = stat.tile([128, HALFS], F32)
            nc.vector.tensor_tensor(c1[:], a0[:], qmaxf[:], OP.mult)
            nc.vector.tensor_tensor(c1[:], c1[:], b0[:], OP.add)
            nc.scalar.activation(c1[:], c1[:], AF.Abs)
            c2 = stat.tile([128, HALFS], F32)
            nc.vector.tensor_tensor(c2[:], a0[:], qminf[:], OP.mult)
            nc.vector.tensor_tensor(c2[:], c2[:], b0[:], OP.add)
            nc.scalar.activation(c2[:], c2[:], AF.Abs)
            chr2 = stat.tile([128, HALFS], F32)
            nc.vector.tensor_tensor(chr2[:], c1[:], c2[:], OP.max)
            rr2 = stat.tile([128, 1], F32)
            nc.vector.tensor_tensor(rr2[:], chr2[:, 0:1], chr2[:, 1:2], OP.max)
            r2col = dram.tile([128], F32)
            nc.sync.dma_start(r2col[:], rr2[:])
            r2all = stat.tile([128, 128], F32)
            nc.sync.dma_start(
                r2all[:],
                bass.AP(tensor=r2col.tensor, offset=r2col[:].offset,
                        ap=[[0, 128], [1, 128]]),
            )
            r2 = stat.tile([128, 1], F32)
            nc.vector.tensor_reduce(r2[:], r2all[:], AX.X, OP.max)
            r2m = stat.tile([128, 1], F32)
            nc.vector.tensor_scalar(r2m[:], r2[:], 1e-30, None, OP.max)
            s2_b, bwb2 = _bitexp_pow2(nc, stat, r2m[:], "s2")

            # exp2 = bw2 - 7
            e2i = stat.tile([128, 1], I32)
            nc.vector.tensor_scalar(e2i[:], bwb2[:], 1, -134, OP.mult, OP.add)
            e2f = stat.tile([128, 1], F32)
            nc.vector.tensor_copy(e2f[:], e2i[:])
            nc.sync.dma_start(out_exp[:], e2f[0:1, 0:1])

            # A' = A0*s2, B' = B0*s2
            ap_ = stat.tile([128, HALFS], F32)
            nc.scalar.activation(ap_[:], a0[:], AF.Copy, scale=s2_b[:, 0:1])
            bp_ = stat.tile([128, HALFS], F32)
            nc.scalar.activation(bp_[:], b0[:], AF.Copy, scale=s2_b[:, 0:1])

            # ---- phase 3: out = int8(relu(A'*q + B')) on ACT, DMA out per chunk ----
            for h in range(HALFS):
                ah, bh = ap_[:, h : h + 1], bp_[:, h : h + 1]
                for i in range(NIMG):
                    c0 = _col(h, i)
                    nc.scalar.activation(
                        o_sb[:, c0 : c0 + PX], q_sb[:, c0 : c0 + PX],
                        AF.Relu, bias=bh, scale=ah,
                    )
                    nc.sync.dma_start(
                        out_val[i, h * 128 : (h + 1) * 128, :],
                        o_sb[:, c0 : c0 + PX],
                    )

    nc.finalize()
    return nc


def _get_nc():
    if "nc" not in _cached:
        _cached["nc"] = _build()
    return _cached["nc"]


def kernel(x_val, x_exp, w_val, w_exp, gamma, beta, _trace=False):
    nc = _get_nc()

    bf16 = ml_dtypes.bfloat16
    x = np.asarray(x_val).reshape(N, CIN, H * W).astype(bf16)
    # weights: [COUT, CIN, KH, KW] -> [KH*KW, CIN, COUT]
    w = np.ascontiguousarray(
        np.asarray(w_val).astype(np.float32).transpose(2, 3, 1, 0).reshape(KH * KW, CIN, COUT)
    ).astype(bf16)
    sxw = np.array([[np.float32(x_exp) + np.float32(w_exp)]], dtype=np.float32)
    g2 = np.ascontiguousarray(np.asarray(gamma, np.float32).reshape(HALFS, 128))
    b2 = np.ascontiguousarray(np.asarray(beta, np.float32).reshape(HALFS, 128))

    in_maps = []
    for c in range(NCORES):
        in_maps.append({
            "x": np.ascontiguousarray(x[c * NIMG : (c + 1) * NIMG]),
            "w": w,
            "scal": sxw,
            "gamma2": g2,
            "beta2": b2,
        })

    res = run_bass_kernel_spmd(nc, in_maps, list(range(NCORES)), trace=_trace)
    out = np.concatenate([res.results[c]["out_val"] for c in range(NCORES)], axis=0)
    out = out.reshape(N, COUT, OH, OW)
    exp2 = np.float32(res.results[0]["out_exp"][0, 0])
    if _trace:
        kernel.last_results = res
    return out, exp2


# revision 20
# speedup vs baseline: 1.2709x; 1.1509x over previous
"""Trainium2 Bass kernel for nn_BasicConv2d (int8 conv + global requant + BN + requant + ReLU).

Self-contained: takes full inputs, shards batch dim over 8 NeuronCores,
runs one SPMD Bass program (conv as 9 shifted matmuls, tiny AllGathers for
the global max / BN-stat reductions), gathers full output.
"""
import numpy as np
import ml_dtypes

import jax  # noqa: F401  (axon PJRT backend provides the 8 NeuronCores)

try:
    jax.config.update("jax_compilation_cache_dir", "/tmp/jaxcache")
    jax.config.update("jax_persistent_cache_min_compile_time_secs", 0.0)
except Exception:
    pass

import concourse.bass as bass
import concourse.bass_isa as bass_isa
import concourse.tile as tile
from concourse import mybir, bacc
from concourse.bass_utils import run_bass_kernel_spmd

F32 = mybir.dt.float32
I32 = mybir.dt.int32
I8 = mybir.dt.int8
BF16 = mybir.dt.bfloat16
AF = mybir.ActivationFunctionType
OP = mybir.AluOpType
AX = mybir.AxisListType

N, CIN, H, W = 32, 128, 56, 56
COUT, KH, KW = 256, 3, 3
OH, OW = 54, 54
PX = OH * OW            # 2916
NCORES = 8
NIMG = N // NCORES      # 4 images per core
NRB = 6                 # row blocks per image (9 output rows each)
RBPX = PX // NRB        # 486 = 9 rows * 54 cols
HALFS = 2               # two 128-channel halves of COUT
COLS_H = NIMG * PX      # 11664 columns per half
COLS = HALFS * COLS_H   # 23328
EPS = 1e-5
RG = [list(range(NCORES))]
CC1 = 520               # [0:256)=chmax, [256:512)=chmin, [512]=local r1
# phase-3 column split per (half, image) chunk of 2916
P3_ACT, P3_DVE, P3_GPS = 1600, 816, 500

_cached = {}


def _col(h, i, rb=0):
    return (h * NIMG + i) * PX + rb * RBPX


def _bitexp_pow2(nc, pool, r_ap, name, p=128):
    """r [p,1] f32 (>0) -> (s [p,1] f32 = 2^(7-ceil(log2 r)),
    bwb [p,1] i32 = ceil(log2 r) + 127). Exact bit arithmetic."""
    ri = r_ap.bitcast(I32)
    eb = pool.tile([p, 1], I32, tag=f"{name}_eb")
    nc.vector.tensor_scalar(eb[:], ri, 23, 0xFF, OP.logical_shift_right, OP.bitwise_and)
    mant = pool.tile([p, 1], I32, tag=f"{name}_mant")
    nc.vector.tensor_scalar(mant[:], ri, 0x7FFFFF, None, OP.bitwise_and)
    nz = pool.tile([p, 1], I32, tag=f"{name}_nz")
    nc.vector.tensor_scalar(nz[:], mant[:], 0, None, OP.is_gt)
    bwb = pool.tile([p, 1], I32, tag=f"{name}_bwb")
    nc.vector.tensor_tensor(bwb[:], eb[:], nz[:], OP.add)
    t = pool.tile([p, 1], I32, tag=f"{name}_t")
    nc.vector.tensor_scalar(t[:], bwb[:], -1, 261, OP.mult, OP.add)  # 261 - bwb
    sb = pool.tile([p, 1], I32, tag=f"{name}_sb")
    nc.vector.tensor_scalar(sb[:], t[:], 23, None, OP.logical_shift_left)
    s = pool.tile([p, 1], F32, tag=f"{name}_s")
    nc.vector.tensor_copy(s[:], sb[:].bitcast(F32))
    return s, bwb


def _pow2_from_int(nc, pool, oi_ap, name, p=128):
    """2^k for k given as [p,1] int32 (normal range)."""
    b = pool.tile([p, 1], I32, tag=f"{name}_b")
    nc.vector.tensor_scalar(b[:], oi_ap, 127, None, OP.add)
    bs = pool.tile([p, 1], I32, tag=f"{name}_bs")
    nc.vector.tensor_scalar(bs[:], b[:], 23, None, OP.logical_shift_left)
    pt = pool.tile([p, 1], F32, tag=f"{name}_p")
    nc.vector.tensor_copy(pt[:], bs[:].bitcast(F32))
    return pt


def _build():
    nc = bacc.Bacc("TRN2", target_bir_lowering=False, debug=False, num_devices=NCORES)

    x_in = nc.dram_tensor("x", [NIMG, CIN, H * W], BF16, kind="ExternalInput")
    w_in = nc.dram_tensor("w", [KH * KW, CIN, COUT], BF16, kind="ExternalInput")
    scal_in = nc.dram_tensor("scal", [1, 1], F32, kind="ExternalInput")  # x_exp+w_exp
    gamma_in = nc.dram_tensor("gamma2", [HALFS, 128], F32, kind="ExternalInput")
    beta_in = nc.dram_tensor("beta2", [HALFS, 128], F32, kind="ExternalInput")
    out_val = nc.dram_tensor("out_val", [NIMG, COUT, PX], I8, kind="ExternalOutput")
    out_exp = nc.dram_tensor("out_exp", [1, 1], F32, kind="ExternalOutput")

    with tile.TileContext(nc) as tc:
        with (
            tc.tile_pool(name="big", bufs=1) as big,
            tc.tile_pool(name="stat", bufs=1) as stat,
            tc.tile_pool(name="dram", bufs=1, space="DRAM") as dram,
            tc.tile_pool(name="psum", bufs=2, space="PSUM") as psum_pool,
        ):
            # ---- PE warmup: dummy matmuls on never-written SBUF, overlaps input DMA
            dummy = big.tile([128, 128], BF16)
            nc.vector.memset(dummy[:], 1.0)
            wps = psum_pool.tile([128, 128], F32, tag="ps0")
            for _ in range(36):
                nc.tensor.matmul(wps[:], dummy[:], dummy[:], start=True, stop=True)

            # ---- load inputs to SBUF ----
            x_sb = big.tile([128, NIMG, H * W], BF16)
            for i in range(NIMG):
                nc.sync.dma_start(x_sb[:, i, :], x_in[i])
            w_sb = big.tile([128, KH * KW, COUT], BF16)
            nc.sync.dma_start(w_sb[:], w_in[:].rearrange("k p c -> p k c"))
            gam_sb = stat.tile([128, HALFS], F32)
            nc.sync.dma_start(gam_sb[:], gamma_in[:].rearrange("h p -> p h"))
            bet_sb = stat.tile([128, HALFS], F32)
            nc.sync.dma_start(bet_sb[:], beta_in[:].rearrange("h p -> p h"))
            scal_sb = stat.tile([128, 1], F32)
            nc.sync.dma_start(
                scal_sb[:],
                bass.AP(tensor=scal_in, offset=0, ap=[[0, 128], [1, 1]]),
            )

            acc_sb = big.tile([128, COLS], F32)
            q_sb = big.tile([128, COLS], I8)
            o_sb = big.tile([128, COLS], I8)

            mx_raw = stat.tile([128, HALFS, NIMG * NRB], F32)
            mn_raw = stat.tile([128, HALFS, NIMG * NRB], F32)

            # ---- phase 1: conv; G row-blocks share one weight load per k ----
            G = 3
            for i in range(NIMG):
                x_img = x_sb[:, i, :].rearrange("p (r c) -> p r c", c=W)
                for rbg in range(0, NRB, G):
                    for h in range(HALFS):
                        pss = []
                        for g in range(G):
                            ps = psum_pool.tile([128, RBPX], F32, tag=f"ps{g}")
                            pss.append(ps)
                        for k in range(KH * KW):
                            kh, kw = divmod(k, KW)
                            for g in range(G):
                                rb = rbg + g
                                rhs = x_img[:, rb * 9 + kh : rb * 9 + kh + 9,
                                            kw : kw + OW]
                                nc.tensor.matmul(
                                    pss[g][:],
                                    w_sb[:, k, h * 128 : (h + 1) * 128],
                                    rhs,
                                    start=(k == 0),
                                    stop=(k == KH * KW - 1),
                                )
                        for g in range(G):
                            rb = rbg + g
                            c0 = _col(h, i, rb)
                            nc.scalar.activation(acc_sb[:, c0 : c0 + RBPX],
                                                 pss[g][:], AF.Copy)
                            j = i * NRB + rb
                            nc.vector.tensor_reduce(
                                mx_raw[:, h, j : j + 1], pss[g][:], AX.X, OP.max
                            )
                            nc.vector.tensor_reduce(
                                mn_raw[:, h, j : j + 1], pss[g][:], AX.X, OP.min
                            )

            # per-core per-channel acc max/min, packed [kind(2), half(2)] per col
            chmm = stat.tile([128, 2, HALFS], F32)
            for h in range(HALFS):
                nc.vector.tensor_reduce(chmm[:, 0, h : h + 1], mx_raw[:, h, :],
                                        AX.X, OP.max)
                nc.vector.tensor_reduce(chmm[:, 1, h : h + 1], mn_raw[:, h, :],
                                        AX.X, OP.min)
            # local r1 on all partitions (gpsimd cross-partition absmax)
            rloc = stat.tile([128, 1], F32)
            nc.vector.tensor_reduce(rloc[:], chmm[:], AX.XY, OP.max,
                                    apply_absolute_value=True)
            r1c = stat.tile([128, 1], F32)
            nc.gpsimd.partition_all_reduce(r1c[:], rloc[:], 128,
                                           bass_isa.ReduceOp.max)

            # ---- collective 1: AllGather per-channel acc max/min + local r1 ----
            cc1_in = dram.tile([CC1], F32)
            nc.sync.dma_start(
                bass.AP(tensor=cc1_in.tensor, offset=cc1_in[:].offset,
                        ap=[[1, 128], [256, 2], [128, HALFS]]),
                chmm[:],
            )
            nc.sync.dma_start(cc1_in[512:513], r1c[0:1, 0:1])

            cc1_out = dram.tile([NCORES, CC1], F32)
            nc.gpsimd.collective_compute(
                "AllGather", OP.bypass, replica_groups=RG,
                ins=[cc1_in[:].opt()], outs=[cc1_out[:].opt()],
            )

            # r1 on all partitions: broadcast-read the 8 per-core r1 slots
            r1g = stat.tile([128, NCORES], F32)
            nc.sync.dma_start(
                r1g[:],
                bass.AP(tensor=cc1_out.tensor, offset=cc1_out[:].offset + 512,
                        ap=[[0, 128], [CC1, NCORES]]),
            )
            r1 = stat.tile([128, 1], F32)
            nc.vector.tensor_reduce(r1[:], r1g[:], AX.X, OP.max,
                                    apply_absolute_value=True)
            r1m = stat.tile([128, 1], F32)
            nc.vector.tensor_scalar(r1m[:], r1[:], 1.0, None, OP.max)
            # r1 is integer-valued: ceil(log2 r) = floor(log2(2r-1)) = expfield-127
            t2r = stat.tile([128, 1], F32)
            nc.vector.tensor_scalar(t2r[:], r1m[:], 2.0, -1.0, OP.mult, OP.add)
            bwb1 = stat.tile([128, 1], I32)
            nc.vector.tensor_scalar(bwb1[:], t2r[:].bitcast(I32), 23, 0xFF,
                                    OP.logical_shift_right, OP.bitwise_and)
            s1i = stat.tile([128, 1], I32)
            nc.vector.tensor_scalar(s1i[:], bwb1[:], -1, 261, OP.mult, OP.add)
            nc.vector.tensor_scalar(s1i[:], s1i[:], 23, None, OP.logical_shift_left)
            s1_b = stat.tile([128, 1], F32)
            nc.vector.tensor_copy(s1_b[:], s1i[:].bitcast(F32))

            # oe = (x_exp+w_exp) + bw1 - 7 ;  poe = 2^oe, poe2 = 2^(2*oe)
            sxw_i = stat.tile([128, 1], I32)
            nc.vector.tensor_copy(sxw_i[:], scal_sb[:])
            oe_i = stat.tile([128, 1], I32)
            nc.vector.tensor_scalar(oe_i[:], bwb1[:], 1, -134, OP.mult, OP.add)
            nc.vector.tensor_tensor(oe_i[:], oe_i[:], sxw_i[:], OP.add)
            oe2_i = stat.tile([128, 1], I32)
            nc.vector.tensor_scalar(oe2_i[:], oe_i[:], 2, None, OP.mult)
            poe_b = _pow2_from_int(nc, stat, oe_i[:], "poe")
            poe2_b = _pow2_from_int(nc, stat, oe2_i[:], "poe2")

            # global per-channel acc extremes -> q extremes
            gmm = stat.tile([128, 2 * HALFS, NCORES], F32)
            for kh in range(2 * HALFS):
                nc.sync.dma_start(
                    gmm[:, kh, :],
                    bass.AP(tensor=cc1_out.tensor,
                            offset=cc1_out[:].offset + kh * 128,
                            ap=[[1, 128], [CC1, NCORES]]),
                )
            gchmax = stat.tile([128, HALFS], F32)
            nc.vector.tensor_reduce(gchmax[:], gmm[:, 0:HALFS, :], AX.X, OP.max)
            gchmin = stat.tile([128, HALFS], F32)
            nc.vector.tensor_reduce(gchmin[:], gmm[:, HALFS : 2 * HALFS, :],
                                    AX.X, OP.min)
            qmx8 = stat.tile([128, HALFS], I8)
            nc.scalar.activation(qmx8[:], gchmax[:], AF.Copy, scale=s1_b[:, 0:1])
            qmn8 = stat.tile([128, HALFS], I8)
            nc.scalar.activation(qmn8[:], gchmin[:], AF.Copy, scale=s1_b[:, 0:1])
            qmaxf = stat.tile([128, HALFS], F32)
            nc.vector.tensor_copy(qmaxf[:], qmx8[:])
            qminf = stat.tile([128, HALFS], F32)
            nc.vector.tensor_copy(qminf[:], qmn8[:])

            # ---- phase 2: q = int8(acc * s1) ; bn stats of q ----
            stats6 = stat.tile([128, HALFS, NIMG * NRB, 6], F32)
            for h in range(HALFS):
                for i in range(NIMG):
                    c0 = _col(h, i)
                    nc.scalar.activation(
                        q_sb[:, c0 : c0 + PX], acc_sb[:, c0 : c0 + PX],
                        AF.Copy, scale=s1_b[:, 0:1],
                    )
                    for rb in range(NRB):
                        cb = c0 + rb * RBPX
                        nc.vector.bn_stats(
                            stats6[:, h, i * NRB + rb, :], q_sb[:, cb : cb + RBPX]
                        )
            mv = stat.tile([128, HALFS, 2], F32)
            for h in range(HALFS):
                nc.vector.bn_aggr(mv[:, h, :], stats6[:, h, :, :])
            # pre-scale to xf units (mean *= 2^oe, var *= 2^2oe) off critical path
            for h in range(HALFS):
                nc.scalar.activation(mv[:, h, 0:1], mv[:, h, 0:1], AF.Copy,
                                     scale=poe_b[:, 0:1])
                nc.scalar.activation(mv[:, h, 1:2], mv[:, h, 1:2], AF.Copy,
                                     scale=poe2_b[:, 0:1])

            # ---- collective 2: AllGather per-channel (mean, var) ----
            cc2_in = dram.tile([2 * HALFS * 128], F32)
            for h in range(HALFS):
                nc.sync.dma_start(cc2_in[h * 128 : (h + 1) * 128], mv[:, h, 0:1])
                nc.sync.dma_start(
                    cc2_in[256 + h * 128 : 256 + (h + 1) * 128], mv[:, h, 1:2]
                )
            cc2_out = dram.tile([NCORES, 2 * HALFS * 128], F32)
            nc.gpsimd.collective_compute(
                "AllGather", OP.bypass, replica_groups=RG,
                ins=[cc2_in[:].opt()], outs=[cc2_out[:].opt()],
            )
            gmv = stat.tile([128, 2 * HALFS, NCORES], F32)
            for kh in range(2 * HALFS):
                nc.sync.dma_start(
                    gmv[:, kh, :],
                    bass.AP(tensor=cc2_out.tensor,
                            offset=cc2_out[:].offset + kh * 128,
                            ap=[[1, 128], [2 * HALFS * 128, NCORES]]),
                )
            gmean = gmv[:, 0:HALFS, :]
            gvar = gmv[:, HALFS : 2 * HALFS, :]

            # combine: mean_g = avg(mean_i); var_g = avg(var_i + mean_i^2) - mean_g^2
            mean_g = stat.tile([128, HALFS], F32)
            nc.vector.tensor_reduce(mean_g[:], gmean, AX.X, OP.add)
            nc.vector.tensor_scalar(mean_g[:], mean_g[:], 1.0 / NCORES, None, OP.mult)
            m2t = stat.tile([128, HALFS, NCORES], F32)
            nc.vector.tensor_tensor(m2t[:], gmean, gmean, OP.mult)
            nc.vector.tensor_tensor(m2t[:], m2t[:], gvar, OP.add)
            ex2 = stat.tile([128, HALFS], F32)
            nc.vector.tensor_reduce(ex2[:], m2t[:], AX.X, OP.add)
            nc.vector.tensor_scalar(ex2[:], ex2[:], 1.0 / NCORES, None, OP.mult)
            var_g = stat.tile([128, HALFS], F32)
            nc.vector.tensor_tensor(var_g[:], mean_g[:], mean_g[:], OP.mult)
            nc.vector.tensor_tensor(var_g[:], ex2[:], var_g[:], OP.subtract)

            # rs = rsqrt(var_xf + eps), Newton-refined (var already in xf units)
            veps = stat.tile([128, HALFS], F32)
            nc.vector.tensor_scalar(veps[:], var_g[:], EPS, None, OP.add)
            eps_t = stat.tile([128, 1], F32)
            nc.vector.memset(eps_t[:], EPS)
            s_sq = stat.tile([128, HALFS], F32)
            nc.scalar.activation(s_sq[:], var_g[:], AF.Sqrt, bias=eps_t[:, 0:1])
            for it in range(1):
                rcp = stat.tile([128, HALFS], F32, tag=f"rcp{it}")
                nc.vector.reciprocal(rcp[:], s_sq[:])
                tn = stat.tile([128, HALFS], F32, tag=f"tn{it}")
                nc.vector.tensor_tensor(tn[:], veps[:], rcp[:], OP.mult)
                nc.vector.tensor_tensor(tn[:], tn[:], s_sq[:], OP.add)
                nc.vector.tensor_scalar(s_sq[:], tn[:], 0.5, None, OP.mult)
            rs = stat.tile([128, HALFS], F32)
            nc.vector.reciprocal(rs[:], s_sq[:])

            # A0 = 2^oe * rs * gamma ; B0 = beta - mean_g*2^oe * rs*gamma
            rg_t = stat.tile([128, HALFS], F32)
            nc.vector.tensor_tensor(rg_t[:], rs[:], gam_sb[:], OP.mult)
            a0 = stat.tile([128, HALFS], F32)
            nc.scalar.activation(a0[:], rg_t[:], AF.Copy, scale=poe_b[:, 0:1])
            u = stat.tile([128, HALFS], F32)
            nc.vector.tensor_tensor(u[:], mean_g[:], rg_t[:], OP.mult)
            b0 = stat.tile([128, HALFS], F32)
            nc.vector.tensor_tensor(b0[:], bet_sb[:], u[:], OP.subtract)

            # r2 = max_c max(|A0*qmax+B0|, |A0*qmin+B0|)
            c1 = stat.tile([128, HALFS], F32)
            c2 = stat.tile([128, HALFS], F32)
            for h in range(HALFS):
                nc.scalar.activation(c1[:, h : h + 1], qmaxf[:, h : h + 1], AF.Abs,
                                     bias=b0[:, h : h + 1], scale=a0[:, h : h + 1])
                nc.scalar.activation(c2[:, h : h + 1], qminf[:, h : h + 1], AF.Abs,
                                     bias=b0[:, h : h + 1], scale=a0[:, h : h + 1])
            chr2 = stat.tile([128, HALFS], F32)
            nc.vector.tensor_tensor(chr2[:], c1[:], c2[:], OP.max)
            rr2 = stat.tile([128, 1], F32)
            nc.vector.tensor_tensor(rr2[:], chr2[:, 0:1], chr2[:, 1:2], OP.max)
            r2 = stat.tile([128, 1], F32)
            nc.gpsimd.partition_all_reduce(r2[:], rr2[:], 128,
                                           bass_isa.ReduceOp.max)
            r2m = stat.tile([128, 1], F32)
            nc.vector.tensor_scalar(r2m[:], r2[:], 1e-30, None, OP.max)
            s2_b, bwb2 = _bitexp_pow2(nc, stat, r2m[:], "s2")

            # exp2 = bw2 - 7
            e2i = stat.tile([128, 1], I32)
            nc.vector.tensor_scalar(e2i[:], bwb2[:], 1, -134, OP.mult, OP.add)
            e2f = stat.tile([128, 1], F32)
            nc.vector.tensor_copy(e2f[:], e2i[:])
            nc.sync.dma_start(out_exp[:], e2f[0:1, 0:1])

            # A' = A0*s2, B' = B0*s2
            ap_ = stat.tile([128, HALFS], F32)
            nc.scalar.activation(ap_[:], a0[:], AF.Copy, scale=s2_b[:, 0:1])
            bp_ = stat.tile([128, HALFS], F32)
            nc.scalar.activation(bp_[:], b0[:], AF.Copy, scale=s2_b[:, 0:1])

            # ---- phase 3: out = int8(relu(A'*q + B')) on ACT, DMA out per chunk ----
            for h in range(HALFS):
                ah, bh = ap_[:, h : h + 1], bp_[:, h : h + 1]
                for i in range(NIMG):
                    c0 = _col(h, i)
                    nc.scalar.activation(
                        o_sb[:, c0 : c0 + PX], q_sb[:, c0 : c0 + PX],
                        AF.Relu, bias=bh, scale=ah,
                    )
                    nc.sync.dma_start(
                        out_val[i, h * 128 : (h + 1) * 128, :],
                        o_sb[:, c0 : c0 + PX],
                    )

    nc.finalize()
    _dedupe_ldweights(nc)
    return nc


def _dedupe_ldweights(nc):
    """Drop InstLdweights that reload the exact weights already resident in
    the PE array (Bacc emits one per matmul; G row-blocks share weights).
    Waits on a dropped load migrate to the next PE matmul."""
    total = 0
    for func in nc.m.functions:
        for bb in func.blocks:
            keep = []
            last_key = None
            pending_waits = []
            dropped = 0
            for ins in bb.instructions:
                if isinstance(ins, mybir.InstLdweights):
                    a = ins.ins[0]
                    key = (getattr(a, "memref", None), getattr(a, "offset", None),
                           str(getattr(a, "ap", None)), str(getattr(a, "dtype", None)))
                    si = ins.sync_info
                    ups = si.on_update if si else []
                    if key == last_key and key[0] is not None and not ups:
                        if si and si.on_wait:
                            pending_waits.extend(si.on_wait)
                        dropped += 1
                        continue  # drop this instruction
                    last_key = key
                elif isinstance(ins, mybir.InstMatmult):
                    if pending_waits:
                        si = ins.sync_info
                        if si is None:
                            ins.sync_info = mybir.SyncInfo(
                                on_wait=list(pending_waits), on_update=[])
                        else:
                            si.on_wait = list(si.on_wait) + pending_waits
                        pending_waits = []
                elif getattr(ins, "engine", None) == mybir.EngineType.PE:
                    last_key = None  # unknown PE op: don't reuse across it
                keep.append(ins)
            if dropped:
                assert not pending_waits, "dangling waits from dropped ldweights"
                del bb.instructions[:]
                for i in keep:
                    bb.instructions.append(i)
                total += dropped
    return total


def _get_nc():
    if "nc" not in _cached:
        _cached["nc"] = _build()
    return _cached["nc"]


def kernel(x_val, x_exp, w_val, w_exp, gamma, beta, _trace=False):
    nc = _get_nc()

    bf16 = ml_dtypes.bfloat16
    x = np.asarray(x_val).reshape(N, CIN, H * W).astype(bf16)
    # weights: [COUT, CIN, KH, KW] -> [KH*KW, CIN, COUT]
    w = np.ascontiguousarray(
        np.asarray(w_val).astype(np.float32).transpose(2, 3, 1, 0).reshape(KH * KW, CIN, COUT)
    ).astype(bf16)
    sxw = np.array([[np.float32(x_exp) + np.float32(w_exp)]], dtype=np.float32)
    g2 = np.ascontiguousarray(np.asarray(gamma, np.float32).reshape(HALFS, 128))
    b2 = np.ascontiguousarray(np.asarray(beta, np.float32).reshape(HALFS, 128))

    in_maps = []
    for c in range(NCORES):
        in_maps.append({
            "x": np.ascontiguousarray(x[c * NIMG : (c + 1) * NIMG]),
            "w": w,
            "scal": sxw,
            "gamma2": g2,
            "beta2": b2,
        })

    res = run_bass_kernel_spmd(nc, in_maps, list(range(NCORES)), trace=_trace)
    out = np.concatenate([res.results[c]["out_val"] for c in range(NCORES)], axis=0)
    out = out.reshape(N, COUT, OH, OW)
    exp2 = np.float32(res.results[0]["out_exp"][0, 0])
    if _trace:
        kernel.last_results = res
    return out, exp2
